# revision 17
# baseline (speedup 1.0000x reference)
"""Trainium2 Bass kernel for nn_CoefficientDecoder.

reference computation (all f32):
    h = relu(x @ W1.T + b1)         x:[B,256] -> h:[B,64]
    h = h @ Wd3.T + bd3             [B,64]
    h = h @ Wd2.T + bd2             [B,64]
    h = h @ Wd1.T + bd1             [B,64]
    z = h @ W2.T + b2               [B,512]
    out = z @ bases                 bases:[512,4096] -> out:[B,4096]

Strategy: pure data-parallel over the batch dim across 8 NeuronCores
(B=8192 -> 1024 rows/core); weights + bases replicated per core.

Per-core kernel works in "transposed activation" space: the host passes
xT = x_shard.T so every matmul has the contraction dim on partitions with
weights stationary and the batch streaming as the moving operand.  All
matmuls use the fp32r (replicated-fp32) PE mode: 1 cycle/row for moving
dims >= 256 (4x faster than plain fp32) at ~1.5e-4 element precision;
the big GEMM can optionally run in fp16.  Walrus requires fp32r matmul
operands to be produced by a rounding op, so DMA-loaded fp32 tiles pass
through a DVE copy into fp32r tiles, and ACT writes h/zT directly as
fp32r.

All small constants (weights + biases) are packed host-side into one
[128, 840] tensor -> a single DMA instead of 10 (each DMA pays ~1.3 us
issue latency on the queue, which showed up as a 14 us PE startup stall).

    MLP:  hT = W1 @ xT (K=256, 2 acc steps) -> relu+bias on ACT
          dec3/dec2/dec1: 64x64 matmuls, bias via ACT Identity
          zT[512,1024] = W2 @ hT, bias fused into the PSUM->SBUF copy
    GEMM: out[mm*128:+128, s*512:+512] = sum_c zT[c].T @ bases[c, s-chunk]
          s-outer loop order so each seq-block only needs its own bases
          tile (bases DMAs stream in behind the compute); 64 output
          tiles/core, 4 matmuls each, DVE/ACT copy to SBUF, stores
          alternate between the SP and ACT HWDGE queues.

`repeat` wraps the whole body in a hardware For_i loop — used only for
timing (amortizes the ~100 ms axon dispatch overhead).
"""

import numpy as np

import concourse.bass as bass
import concourse.tile as tile
from concourse import bacc, mybir
from concourse.bass import ts
from concourse.bass_utils import run_bass_kernel_spmd

N_CORES = 8
B, IN_F, HID, NB, SEQ = 8192, 256, 64, 512, 4096
B_LOC = B // N_CORES            # 1024 batch rows per core

F32 = mybir.dt.float32
F32R = mybir.dt.float32r
F16 = mybir.dt.float16

# packed-constant column layout (fp32 columns in the [128, NCONST] tensor).
# The three dec layers are linear, so they are folded host-side into W2:
#   W2eff = W2@Wd1@Wd2@Wd3,  b2eff = b2 + (bd3@Wd2.T@Wd1.T + bd2@Wd1.T + bd1)@W2.T
C_W1K0, C_W1K1 = 0, 64
C_W2 = 128            # [64, 512] on partitions 0..63
C_B2P = 640           # [128, 4]
C_B1 = 644
NCONST = 645
NWROUND = 640         # leading region that gets rounded to fp32r

# dtype knobs: ("f32r"|"f16") for the big GEMM operands, ("f32"|"f16") output
GEMM_MODE = "f32r"
OUT_MODE = "f32"

_CACHE = {}


def _build(gemm_mode: str, out_mode: str, repeat: int = 1):
    gemm_dt = F32R if gemm_mode == "f32r" else F16    # on-chip GEMM operand dtype
    bases_dram_dt = F32 if gemm_mode == "f32r" else F16
    out_dt = F32 if out_mode == "f32" else F16

    nc = bacc.Bacc(
        "TRN2",
        target_bir_lowering=False,
        debug=False,
        enable_asserts=False,
        num_devices=N_CORES,
    )

    xT_d = nc.declare_dram_parameter("xT", [IN_F, B_LOC], F32, isOutput=False)
    consts_d = nc.declare_dram_parameter("consts", [128, NCONST], F32, isOutput=False)
    bases_d = nc.declare_dram_parameter("bases", [NB, SEQ], bases_dram_dt, isOutput=False)
    out_d = nc.declare_dram_parameter("out", [B_LOC, SEQ], out_dt, isOutput=True)

    KC = IN_F // 128        # 2 k-chunks for layer 1
    ZC = NB // 128          # 4 z-feature chunks
    NJ = B_LOC // 512       # 2 batch chunks for the MLP moving dim
    MM = B_LOC // 128       # 8 batch sub-chunks for the final GEMM
    SC = SEQ // 512         # 8 seq chunks

    relu = mybir.ActivationFunctionType.Relu
    ident = mybir.ActivationFunctionType.Identity
    copyf = mybir.ActivationFunctionType.Copy

    with tile.TileContext(nc) as tc:
        with (
            tc.tile_pool(name="const", bufs=1) as constp,
            tc.tile_pool(name="stage", bufs=2) as stagep,
            tc.tile_pool(name="bases", bufs=1) as basesp,
            tc.tile_pool(name="xz", bufs=1) as xzp,
            tc.tile_pool(name="act", bufs=3) as actp,
            tc.tile_pool(name="outsb", bufs=6) as outsbp,
            tc.tile_pool(name="mlp_ps", bufs=3, space="PSUM") as mlpp,
            tc.tile_pool(name="out_ps", bufs=5, space="PSUM") as outpp,
        ):
            def body():
                # ---- one DMA for every weight/bias, then round weights.
                # First in program order: it is small and gates the MLP ----
                craw = constp.tile([128, NCONST], F32, tag="craw")
                nc.scalar.dma_start(craw[:], consts_d[:])
                crnd = constp.tile([128, NWROUND], F32R, tag="crnd")
                nc.vector.tensor_copy(crnd[:], craw[:, :NWROUND])

                # ---- x load + round, split by k-half so the first MLP
                # matmul only waits for half the transfer ----
                xT_pkn = xT_d.rearrange("(k p) n -> p k n", p=128)
                xf = stagep.tile([128, KC, B_LOC], F32, tag="xstage")
                xT_sb = xzp.tile([128, KC, B_LOC], F32R, tag="xT")
                for k in range(KC):
                    nc.scalar.dma_start(xf[:, k, :], xT_pkn[:, k, :])
                    nc.vector.tensor_copy(xT_sb[:, k, :], xf[:, k, :])

                w1t = (crnd[:, C_W1K0 : C_W1K0 + 64], crnd[:, C_W1K1 : C_W1K1 + 64])
                w2t = crnd[:HID, C_W2 : C_W2 + NB]
                b2p = craw[:, C_B2P : C_B2P + ZC]
                b1 = craw[:HID, C_B1 : C_B1 + 1]

                # ---- bases load on the SP queue: one tile per seq chunk so
                # each final-GEMM s-block only waits for its own chunk ----
                bases_pcn = bases_d.rearrange("(c p) n -> p c n", p=128)
                bases_sb = []
                for s in range(SC):
                    t = basesp.tile([128, ZC, 512], gemm_dt, tag=f"bases{s}")
                    if gemm_mode == "f32r":
                        f = stagep.tile([128, ZC, 512], F32, tag="bstage")
                        nc.sync.dma_start(f[:], bases_pcn[:, :, ts(s, 512)])
                        nc.vector.tensor_copy(t[:], f[:])
                    else:
                        nc.sync.dma_start(t[:], bases_pcn[:, :, ts(s, 512)])
                    bases_sb.append(t)

                # ---- MLP: produce zT [feature-part, ZC, batch] ----
                zT_sb = xzp.tile([128, ZC, B_LOC], gemm_dt, tag="zT")
                for j in range(NJ):
                    hp = mlpp.tile([HID, 512], F32, tag="mlp")
                    for k in range(KC):
                        nc.tensor.matmul(
                            hp[:],
                            w1t[k],
                            xT_sb[:, k, ts(j, 512)],
                            start=(k == 0),
                            stop=(k == KC - 1),
                        )
                    h = actp.tile([HID, 512], F32R, tag="h")
                    nc.scalar.activation(h[:], hp[:], relu, bias=b1)

                    for c in range(ZC):
                        zp = mlpp.tile([128, 512], F32, tag="mlp")
                        nc.tensor.matmul(
                            zp[:], w2t[:, ts(c, 128)], h[:], start=True, stop=True
                        )
                        nc.scalar.activation(
                            zT_sb[:, c, ts(j, 512)], zp[:], ident,
                            bias=b2p[:, c : c + 1],
                        )

                # ---- final GEMM: out = z @ bases (s outer: each block only
                # needs bases chunk s, which streams in behind compute) ----
                for s in range(SC):
                    for mm_i in range(MM):
                        op = outpp.tile([128, 512], F32, tag="op")
                        for c in range(ZC):
                            nc.tensor.matmul(
                                op[:],
                                zT_sb[:, c, ts(mm_i, 128)],
                                bases_sb[s][:, c, :],
                                start=(c == 0),
                                stop=(c == ZC - 1),
                            )
                        ob = outsbp.tile([128, 512], out_dt, tag="ob")
                        if (mm_i + s) % 2 == 0:
                            nc.vector.tensor_copy(ob[:], op[:])
                        else:
                            nc.scalar.activation(ob[:], op[:], copyf)
                        dma_eng = nc.sync if (mm_i % 2 == 0) else nc.scalar
                        dma_eng.dma_start(out_d[ts(mm_i, 128), ts(s, 512)], ob[:])

            if repeat == 1:
                body()
            else:
                with tc.For_i(0, repeat, 1):
                    body()

    nc.compile()
    return nc


def _get_nc(gemm_mode: str, out_mode: str, repeat: int = 1):
    key = (gemm_mode, out_mode, repeat)
    if key not in _CACHE:
        _CACHE[key] = _build(gemm_mode, out_mode, repeat)
    return _CACHE[key]


def _pack_consts(W1, b1, Wd1, bd1, Wd2, bd2, Wd3, bd3, W2, b2):
    W2eff = W2 @ Wd1 @ Wd2 @ Wd3                      # [512, 64]
    b2eff = b2 + (bd3 @ Wd2.T @ Wd1.T + bd2 @ Wd1.T + bd1) @ W2.T
    c = np.zeros((128, NCONST), np.float32)
    W1T = W1.T  # [256, 64]
    c[:, C_W1K0 : C_W1K0 + 64] = W1T[:128]
    c[:, C_W1K1 : C_W1K1 + 64] = W1T[128:]
    c[:HID, C_W2 : C_W2 + NB] = W2eff.T
    c[:, C_B2P : C_B2P + NB // 128] = b2eff.reshape(NB // 128, 128).T
    c[:HID, C_B1] = b1
    return c


def _in_maps(x, W1, b1, Wd1, bd1, Wd2, bd2, Wd3, bd3, W2, b2, bases, gemm_mode):
    bases_np = np.float32 if gemm_mode == "f32r" else np.float16
    common = {
        "consts": _pack_consts(W1, b1, Wd1, bd1, Wd2, bd2, Wd3, bd3, W2, b2),
        "bases": np.ascontiguousarray(bases.astype(bases_np)),
    }
    maps = []
    for i in range(N_CORES):
        m = dict(common)
        m["xT"] = np.ascontiguousarray(x[i * B_LOC : (i + 1) * B_LOC].T)
        maps.append(m)
    return maps


def run(inputs: dict, gemm_mode: str = GEMM_MODE, out_mode: str = OUT_MODE,
        repeat: int = 1, **run_kwargs):
    """Shard, execute on 8 cores, gather. Returns (out, BassKernelResults)."""
    nc = _get_nc(gemm_mode, out_mode, repeat)
    in_maps = _in_maps(**{k: np.asarray(v) for k, v in inputs.items()}, gemm_mode=gemm_mode)
    res = run_bass_kernel_spmd(nc, in_maps, list(range(N_CORES)), **run_kwargs)
    shards = [np.asarray(res.results[i]["out"], dtype=np.float32) for i in range(N_CORES)]
    out = np.concatenate(shards, axis=0)
    return out, res


def kernel(**inputs) -> np.ndarray:
    out, _ = run(inputs)
    return out
